# revision 40
# baseline (speedup 1.0000x reference)
"""Trainium2 Bass kernel for nn_Attention_50964081935360.

Single-query attention with a global-Frobenius-norm score scale:
  scores[b,s] = key[b,s,:] . query[b,:]
  denom      = ||key||_F  (over the WHOLE key tensor, all batches)
  p          = softmax(scores/denom) masked to s < seq_lens[b], renormalized
  out        = p[..., None] + 1e-15

Sharding: data-parallel over batch B=32 across 8 NeuronCores (4 batches per
core). Cross-core communication is a TWO-STAGE scalar AllReduce of the key
shard's sum of squares: AR1 covers super-tiles 0..14 and launches ~94% into
the DMA stream, absorbing the (large, variable) inter-core start skew under
the stream; AR2 covers just the last super-tile and costs only ~8 us of
mesh latency after the cores are aligned.

Per-core plan (memory-bound; key shard is 64 MiB, HBM floor ~187 us/core):
  - 15x 4 MiB super-tile DMAs on the sync HWDGE ring, one dma_start each,
    laid out [p, (j d)] with s = 1024g + 8p + j so each partition reads
    32 KiB CONTIGUOUS (~338 GB/s sustained); last super-tile in 4x 1 MiB
    chunks to shorten the post-stream drain.
  - DVE: affine_mul_reduce per s-tile column for scores; ACT: Square+accum
    chunks for the local ssq; TensorE all-ones matmuls do every partition
    reduce/broadcast (gpsimd cross-lane ops stall ~6 us on pool config).
  - AR1 (super-tiles 0..14) fires ~94% into the stream so its completion
    aligns the cores right before AR2 — placing it earlier makes AR2
    re-pay the start skew. AR2 then costs only ~8 us of mesh latency.
    Both cc_in DMAs ride the scalar ring behind their ACT producers (a
    sync-ring trigger's sem wait would stall key-load triggers).
  - inv = rsqrt(global ssq) via DVE bit-trick + 2 Newton steps; the Exp
    ACT table is preloaded during AR2, so zero table switches post-AR.
  - Epilogue fused across batches: one exp (per-partition scale), one
    masked multiply, one [p, b, t]-view column reduce, PE partition sum,
    one strided output DMA (no on-chip transposes).
  A warm-up AllReduce at kernel start pays the ncfw wakeup latency.
"""

import sys

import numpy as np

if "/opt/trn_rl_repo" not in sys.path:
    sys.path.insert(0, "/opt/trn_rl_repo")

import concourse.bacc as bacc
import concourse.bass as bass
import concourse.mybir as mybir
import concourse.tile as tile
from concourse.bass_isa import ReduceOp
from concourse.bass_utils import run_bass_kernel_spmd

B, S, D = 32, 4096, 1024
NCORES = 8
BPC = B // NCORES  # batches per core
P = 128            # s-tile partition size
NT = S // P        # s-tiles per batch (32)
NC_TILES = BPC * NT  # tiles per core (128)
PERTURB = 1e-15

F32 = mybir.dt.float32
I32 = mybir.dt.int32
ALU = mybir.AluOpType
ACTF = mybir.ActivationFunctionType

SUB = 8        # s-tiles per key super-tile
NG = NT // SUB  # super-tiles per batch (4)
KEY_BUFS = 4   # in-flight key super-tiles (4 MiB each)
NSQ = 4        # ACT square ops per super-tile (PSUM tile = 8 KiB/partition)


def build() -> bass.Bass:
    nc = bacc.Bacc(
        "TRN2", target_bir_lowering=False, debug=False, num_devices=NCORES
    )
    key_ext = nc.declare_dram_parameter("key", [BPC, S, D], F32, isOutput=False)
    q_ext = nc.declare_dram_parameter("query", [BPC, D], F32, isOutput=False)
    sl_ext = nc.declare_dram_parameter("seq_lens", [1, BPC], I32, isOutput=False)
    out_ext = nc.declare_dram_parameter("out", [BPC, S, 1], F32, isOutput=True)

    # Collective bounce buffers (internal DRAM; output must be Shared).
    # Two-stage ssq AllReduce: AR1 covers super-tiles 0..14 and launches
    # ~94% into the stream (absorbing inter-core skew under the stream);
    # AR2 covers only the last super-tile and pays just mesh latency.
    cc_in = nc.dram_tensor("cc_in", [1, 1], F32)
    cc_out = nc.dram_tensor("cc_out", [1, 1], F32, addr_space="Shared")
    cc_in2 = nc.dram_tensor("cc_in2", [1, 1], F32)
    cc_out2 = nc.dram_tensor("cc_out2", [1, 1], F32, addr_space="Shared")
    # Dummy collective buffers: a warm-up AllReduce at kernel start pays the
    # ncfw wakeup latency so the real one at the end doesn't.
    ccw_in = nc.dram_tensor("ccw_in", [1, 1], F32)
    ccw_out = nc.dram_tensor("ccw_out", [1, 1], F32, addr_space="Shared")

    key_ap = key_ext.ap()
    out_ap = out_ext.ap()

    with tile.TileContext(nc) as tc:
        with (
            tc.tile_pool(name="keys", bufs=KEY_BUFS) as kpool,
            tc.tile_pool(name="amr_scratch", bufs=4) as amrpool,
            tc.tile_pool(name="sq_psum", bufs=1, space="PSUM") as sqpool,
            tc.tile_pool(name="mm_psum", bufs=1, space="PSUM") as psmall,
            tc.tile_pool(name="persist", bufs=1) as pp,
        ):
            # all-ones stationaries for TensorE partition reduce/broadcast
            # (PE is otherwise idle; avoids slow gpsimd cross-lane ops)
            ones_full = pp.tile([P, P], F32)
            nc.vector.memset(ones_full[:, :], 1.0)
            ones_row = pp.tile([1, P], F32)
            nc.vector.memset(ones_row[:, :], 1.0)
            # ---- setup: query broadcast, seq_lens, s-index ----
            # q/seq_lens ride the ACT HWDGE ring so they don't queue behind
            # the 512 KiB key loads on the sync ring.
            # q DMAs go FIRST on the sync ring (HWDGE FIFO per ring), so they
            # land before the 512 KiB key-load flood; batch 0's broadcast
            # alone gates the first AMR.
            def load_supertile(b, g):
                # one 4 MiB DMA per super-tile with CONTIGUOUS 32 KiB per
                # partition: tile[p, j*D+d] = key[b, g*SUB*P + p*SUB + j, d],
                # i.e. column block j holds s = g*SUB*P + SUB*p + j. 128
                # sequential 32 KiB descriptors keep HBM near line rate.
                # All key loads ride nc.sync: HWDGE triggers on the scalar
                # ring would queue behind ACT squares and starve the stream.
                kt = kpool.tile([P, SUB * D], F32, tag="key")
                src = key_ap[
                    b, g * SUB * P : (g + 1) * SUB * P, :
                ].rearrange("(p j) d -> p j d", p=P)
                dst = kt[:, :].rearrange("p (j d) -> p j d", d=D)
                nc.sync.dma_start(out=dst, in_=src)
                return kt

            q_tiles = []
            for b in range(BPC):
                qr = pp.tile([P, D], F32, tag=f"qrep{b}")
                # scalar (ACT) ring: empty at kernel start, so these tiny
                # loads don't delay the first key DMA trigger on sync.
                nc.scalar.dma_start(
                    out=qr[0:1, :], in_=q_ext.ap()[b : b + 1, :]
                )
                q_tiles.append(qr)
            for b in range(BPC):
                nc.gpsimd.partition_broadcast(q_tiles[b][:, :], q_tiles[b][0:1, :])
            q_rep = [q_tiles[b][:, :] for b in range(BPC)]

            # warm-up collective (result unused)
            warm = pp.tile([1, 1], F32)
            nc.vector.memset(warm[:, :], 0.0)
            nc.scalar.dma_start(out=ccw_in.ap()[:, :], in_=warm[:, :])
            nc.gpsimd.collective_compute(
                "AllReduce",
                ALU.add,
                replica_groups=[list(range(NCORES))],
                ins=[ccw_in.ap().opt()],
                outs=[ccw_out.ap().opt()],
            )

            sl_i = pp.tile([1, BPC], I32)
            nc.scalar.dma_start(out=sl_i[:, :], in_=sl_ext.ap()[:, :])
            sl_f = pp.tile([P, BPC], F32)
            nc.vector.tensor_copy(out=sl_f[0:1, :], in_=sl_i[:, :])
            nc.gpsimd.partition_broadcast(sl_f[:, :], sl_f[0:1, :])

            # s_idx[p, c=(g,j)] = SUB*p + SUB*P*g + j  (sequence position of
            # scores[p, c] under the contiguous-per-partition key layout)
            s_idx_i = pp.tile([P, NT], I32)
            nc.gpsimd.iota(
                s_idx_i[:, :],
                pattern=[[SUB * P, NG], [1, SUB]],
                base=0,
                channel_multiplier=SUB,
            )
            s_idx = pp.tile([P, NT], F32)
            nc.vector.tensor_copy(out=s_idx[:, :], in_=s_idx_i[:, :])

            # masks depend only on s_idx/seq_lens: compute them up front so
            # the post-AllReduce tail is shorter
            masks_all = pp.tile([P, BPC * NT], F32)
            for b in range(BPC):
                nc.vector.tensor_scalar(
                    out=masks_all[:, b * NT : (b + 1) * NT],
                    in0=s_idx[:, :],
                    scalar1=sl_f[:, b : b + 1],
                    scalar2=None,
                    op0=ALU.is_lt,
                )

            # ---- main streaming loop over key super-tiles ----
            scores = pp.tile([P, NC_TILES], F32)
            ssqcols = pp.tile([P, NSQ * BPC * NG], F32)

            for b in range(BPC):
                for g in range(NG):
                    if b == BPC - 1 and g == NG - 1:
                        continue  # last super-tile handled below, split fine
                    kt = load_supertile(b, g)
                    # scores columns: sum_d key*q (one DVE pass per s-tile)
                    for j in range(SUB):
                        c = b * NT + g * SUB + j
                        amr = amrpool.tile([P, D], F32, tag="amr")
                        nc.vector.affine_mul_reduce(
                            out=amr[:, :],
                            accum_out=scores[:, c : c + 1],
                            in0=kt[:, j * D : (j + 1) * D],
                            in1=q_rep[b][:, :],
                            scale=1.0,
                            bias=0.0,
                        )
                    # global ssq only needs a total: square+accum over a chunk
                    # of the super-tile per ACT op (amortizes ACT overheads);
                    # out goes to PSUM (unused otherwise, saves SBUF)
                    for h in range(NSQ):
                        c2 = NSQ * (b * NG + g) + h
                        w = SUB * D // NSQ
                        sq = sqpool.tile([P, w], F32, tag="sq")
                        nc.scalar.activation(
                            out=sq[:, :],
                            in_=kt[:, h * w : (h + 1) * w],
                            func=ACTF.Square,
                            accum_out=ssqcols[:, c2 : c2 + 1],
                        )

            # Last super-tile in four 1 MiB chunks so the post-stream compute
            # drain is ~2 AMRs + 1 square instead of a full 4 MiB tile.
            bl, gl = BPC - 1, NG - 1
            full_last = key_ap[
                bl, gl * SUB * P : (gl + 1) * SUB * P, :
            ].rearrange("(p j) d -> p j d", p=P)
            ktcs = []
            for jj in range(4):
                ktc = kpool.tile([P, 2 * D], F32, tag="keyc")
                ktcs.append(ktc)
            for jj in range(4):
                # all four chunk DMAs issue on sync BEFORE the AR1 cc_in
                # DMA below them in this loop would otherwise... (they must
                # precede any sem-waiting trigger in the sync FIFO)
                nc.sync.dma_start(
                    out=ktcs[jj][:, :].rearrange("p (j d) -> p j d", d=D),
                    in_=full_last[:, 2 * jj : 2 * jj + 2, :],
                )
            # AR1 chain emitted BEFORE the last super-tile: Tile keeps
            # per-engine emission order, so this DVE reduce runs as soon as
            # super-tiles 0..14's squares land (~94% of stream) and AR1
            # absorbs the inter-core skew under the stream's tail.
            nsplit = NSQ * (BPC * NG - 1)  # cols of super-tiles 0..14
            ssq_r = pp.tile([P, 1], F32)
            nc.vector.tensor_reduce(
                out=ssq_r[:, :], in_=ssqcols[:, 0:nsplit],
                axis=mybir.AxisListType.XYZW, op=ALU.add,
            )
            ssqp = psmall.tile([P, 1], F32, tag="ssqp")
            nc.tensor.matmul(
                ssqp[:, :], ones_full[:, :], ssq_r[:, :], start=True, stop=True
            )
            ssq_sb = pp.tile([1, 1], F32)
            nc.scalar.copy(out=ssq_sb[:, :], in_=ssqp[0:1, :])
            nc.scalar.dma_start(out=cc_in.ap()[:, :], in_=ssq_sb[:, :])
            nc.gpsimd.collective_compute(
                "AllReduce",
                ALU.add,
                replica_groups=[list(range(NCORES))],
                ins=[cc_in.ap().opt()],
                outs=[cc_out.ap().opt()],
            )

            for jj in range(4):
                ktc = ktcs[jj]
                for jc in range(2):
                    c = bl * NT + gl * SUB + 2 * jj + jc
                    amr = amrpool.tile([P, D], F32, tag="amr")
                    nc.vector.affine_mul_reduce(
                        out=amr[:, :],
                        accum_out=scores[:, c : c + 1],
                        in0=ktc[:, jc * D : (jc + 1) * D],
                        in1=q_rep[bl][:, :],
                        scale=1.0,
                        bias=0.0,
                    )
                c2 = NSQ * (bl * NG + gl) + jj
                sq = sqpool.tile([P, 2 * D], F32, tag="sq")
                nc.scalar.activation(
                    out=sq[:, :],
                    in_=ktc[:, :],
                    func=ACTF.Square,
                    accum_out=ssqcols[:, c2 : c2 + 1],
                )

            # ---- local ssq reduction -> scalars, two-stage AllReduce ----
            # DVE free-dim reduce (fast) + TensorE all-ones matmul for the
            # partition reduce — avoids gpsimd CROSS_LANE_REDUCE's ~6 us
            # pool-config setup stall on the critical path.
            ssq_r2 = pp.tile([P, 1], F32)
            nc.vector.tensor_reduce(
                out=ssq_r2[:, :], in_=ssqcols[:, nsplit:],
                axis=mybir.AxisListType.XYZW, op=ALU.add,
            )
            ssqp2 = psmall.tile([P, 1], F32, tag="ssqp2")
            nc.tensor.matmul(
                ssqp2[:, :], ones_full[:, :], ssq_r2[:, :],
                start=True, stop=True,
            )
            ssq_sb2 = pp.tile([1, 1], F32)
            nc.scalar.copy(out=ssq_sb2[:, :], in_=ssqp2[0:1, :])
            nc.scalar.dma_start(out=cc_in2.ap()[:, :], in_=ssq_sb2[:, :])
            nc.gpsimd.collective_compute(
                "AllReduce",
                ALU.add,
                replica_groups=[list(range(NCORES))],
                ins=[cc_in2.ap().opt()],
                outs=[cc_out2.ap().opt()],
            )
            # Preload the Exp ACT table while AR2 is in flight. The inv
            # chain below is DVE-only, so Exp is the only table the tail
            # ever needs — no switches on the critical path.
            tdum = pp.tile([P, 1], F32)
            nc.scalar.activation(out=tdum[:, :], in_=ssqp[:, :], func=ACTF.Exp)

            # broadcast AR1's output to all partitions DURING AR2 (k=1 ones
            # matmul, start of a PSUM accumulation group), then accumulate
            # AR2's output in after it lands — the post-AR2 path only pays
            # one tiny matmul instead of DMA+reduce+matmul.
            ga_sb = pp.tile([1, 1], F32)
            nc.sync.dma_start(out=ga_sb[:, :], in_=cc_out.ap()[:, :])
            gb_sb = pp.tile([1, 1], F32)
            nc.sync.dma_start(out=gb_sb[:, :], in_=cc_out2.ap()[:, :])
            gbp = psmall.tile([P, 1], F32, tag="gbp")
            nc.tensor.matmul(
                gbp[:, :], ones_row[:, :], ga_sb[:, :], start=True, stop=False
            )
            nc.tensor.matmul(
                gbp[:, :], ones_row[:, :], gb_sb[:, :], start=False, stop=True
            )
            # inv = rsqrt(gssq) on DVE only: bit-trick seed + 2 Newton steps
            # (quadratic: ~3.4% -> 1.7e-3 -> 4e-6 rel err). Avoids Sqrt/Ln
            # ACT table loads after the AllReduce.
            g_sb = pp.tile([P, 1], F32)
            nc.vector.tensor_copy(out=g_sb[:, :], in_=gbp[:, :])
            magic = pp.tile([P, 1], I32)
            nc.vector.memset(magic[:, :], 0x5F3759DF)
            halfbits = pp.tile([P, 1], I32)
            nc.vector.tensor_scalar(
                out=halfbits[:, :],
                in0=g_sb[:, :].bitcast(I32),
                scalar1=1,
                scalar2=None,
                op0=ALU.logical_shift_right,
            )
            y_i = pp.tile([P, 1], I32)
            nc.vector.scalar_tensor_tensor(
                out=y_i[:, :],
                in0=magic[:, :],
                scalar=1,
                in1=halfbits[:, :],
                op0=ALU.mult,
                op1=ALU.subtract,
            )
            y = y_i[:, :].bitcast(F32)
            ya = pp.tile([P, 1], F32)
            yb = pp.tile([P, 1], F32)
            inv_rep = pp.tile([P, 1], F32)
            for it, (src, dst) in enumerate([(y, ya[:, :]), (ya[:, :], inv_rep[:, :])]):
                gy2 = pp.tile([P, 1], F32, tag=f"gy2_{it}")
                nc.vector.scalar_tensor_tensor(
                    out=gy2[:, :],
                    in0=src,
                    scalar=g_sb[:, 0:1],
                    in1=src,
                    op0=ALU.mult,
                    op1=ALU.mult,
                )
                corr = yb[:, :]
                nc.vector.tensor_scalar(
                    out=corr,
                    in0=gy2[:, :],
                    scalar1=-0.5,
                    scalar2=1.5,
                    op0=ALU.mult,
                    op1=ALU.add,
                )
                nc.vector.tensor_scalar(
                    out=dst,
                    in0=src,
                    scalar1=corr,
                    scalar2=None,
                    op0=ALU.mult,
                )

            # ---- epilogue: masked softmax, fused across batches ----
            # inv_rep is per-partition (same for every batch), so ONE exp
            # covers all 128 score columns; mask + per-batch column sums are
            # one multiply + one innermost-axis reduce over a [p, b, t] view.
            e_allt = pp.tile([P, NC_TILES], F32)
            nc.scalar.activation(
                out=e_allt[:, :],
                in_=scores[:, :],
                func=ACTF.Exp,
                scale=inv_rep[:, :],
            )
            em_allt = pp.tile([P, NC_TILES], F32)
            nc.vector.scalar_tensor_tensor(
                out=em_allt[:, :],
                in0=e_allt[:, :],
                scalar=1.0,
                in1=masks_all[:, :],
                op0=ALU.mult,
                op1=ALU.mult,
            )
            zcols = pp.tile([P, BPC], F32)
            nc.vector.tensor_reduce(
                out=zcols[:, :].rearrange("p (b o) -> p b o", o=1),
                in_=em_allt[:, :].rearrange("p (b t) -> p b t", t=NT),
                axis=mybir.AxisListType.X,
                op=ALU.add,
            )

            zsum = psmall.tile([P, BPC], F32, tag="zsum")
            nc.tensor.matmul(
                zsum[:, :], ones_full[:, :], zcols[:, :], start=True, stop=True
            )
            invz = pp.tile([P, BPC], F32)
            nc.vector.reciprocal(out=invz[:, :], in_=zsum[:, :])

            # s = SUB*P*g + SUB*p + j: partition p writes SUB contiguous
            # floats per (b, g) — no on-chip transpose, and ONE output DMA
            # for all batches (4 separate DMAs serialize ~800 ns apart on
            # the sync ring's FIFO).
            o_all = pp.tile([P, BPC * NT], F32)
            for b in range(BPC):
                nc.vector.tensor_scalar(
                    out=o_all[:, b * NT : (b + 1) * NT],
                    in0=em_allt[:, b * NT : (b + 1) * NT],
                    scalar1=invz[:, b : b + 1],
                    scalar2=PERTURB,
                    op0=ALU.mult,
                    op1=ALU.add,
                )
            dst = out_ap[:, :, 0].rearrange("b (g p j) -> p b g j", p=P, j=SUB)
            src = o_all[:, :].rearrange("p (b g j) -> p b g j", b=BPC, j=SUB)
            nc.sync.dma_start(out=dst, in_=src)

    nc.compile()
    return nc


_NC_CACHE = None


def _get_nc():
    global _NC_CACHE
    if _NC_CACHE is None:
        _NC_CACHE = build()
    return _NC_CACHE


def make_in_maps(key, query, seq_lens):
    key = np.ascontiguousarray(np.asarray(key, dtype=np.float32))
    query = np.ascontiguousarray(np.asarray(query, dtype=np.float32))
    seq_lens = np.ascontiguousarray(np.asarray(seq_lens, dtype=np.int32))
    in_maps = []
    for c in range(NCORES):
        lo, hi = c * BPC, (c + 1) * BPC
        in_maps.append(
            {
                "key": key[lo:hi],
                "query": query[lo:hi],
                "seq_lens": seq_lens[lo:hi].reshape(1, BPC),
            }
        )
    return in_maps


def kernel(key, query, seq_lens, **run_kwargs):
    nc = _get_nc()
    in_maps = make_in_maps(key, query, seq_lens)
    res = run_bass_kernel_spmd(
        nc, in_maps, core_ids=list(range(NCORES)), **run_kwargs
    )
    outs = [res.results[c]["out"].reshape(BPC, S, 1) for c in range(NCORES)]
    full = np.concatenate(outs, axis=0).astype(np.float32)
    if run_kwargs:
        kernel.last_results = res  # expose profile info to test harness
    return full

